# revision 2
# baseline (speedup 1.0000x reference)
"""DimensionalConsistencyLoss on 8 Trainium2 NeuronCores.

Per core: 1280 constraint slots (10 cols of 128; cols 0-3 pos, 4-7 neg,
8-9 neu). The Pool-sequencer indirect-DMA dispatch (~1.66us per 128-desc
op, serial, measured) is the hard bottleneck; 10 ops is structural
(1 desc per channel per op). Everything else is arranged around it:

  - 8 row-gather ops FIRST (compute pipelines behind them), 2 neu
    element-gather ops LAST (their data feeds the output DMA directly,
    no compute -> shortest tail).
  - Single SWDGE queue; one counting gather semaphore (FIFO completion).
  - Per row tile: ACT Abs+accum -> rowsum, DVE is_equal-extract -> t.
    Both rates (~1.2us / ~0.78us per tile) are under the 1.66us/op
    gather pace.
  - ACT Abs table preloaded by a dummy activation during the idx wait.
  - Device outputs raw [128, 18]: extracted t (8) | neu t (2, straight
    from DMA) | rowsums (8). Host does the O(slots) piecewise finish
    (exact f32, same numerics as the reference).
"""

import numpy as np

import concourse.bacc as bacc
import concourse.bass as bass
import concourse.mybir as mybir
from concourse.bass_utils import run_bass_kernel_spmd

P = 128
VOCAB = 100000
DIM = 512
N_POS = 4096
N_NEG = 4096
N_NEU = 2048
N_ALL = N_POS + N_NEG + N_NEU
N_CORES = 8

SLOTS = N_ALL // N_CORES           # 1280
COLS = SLOTS // P                  # 10
RCOLS = (N_POS + N_NEG) // N_CORES // P   # 8 row columns (pos/neg)
OUTW = COLS + RCOLS                # 18

CONSISTENCY_WEIGHT = 0.5
SPARSITY_WEIGHT = 0.1
C_SP = SPARSITY_WEIGHT / (DIM - 1)

CW_RAMP = DIM                       # coef: ramp | dims
CW_TOT = DIM + RCOLS

F32 = mybir.dt.float32
I32 = mybir.dt.int32
OP = mybir.AluOpType
AF = mybir.ActivationFunctionType

_nc_cache = None


def _build_program():
    global _nc_cache
    if _nc_cache is not None:
        return _nc_cache

    nc = bacc.Bacc(
        "TRN2", target_bir_lowering=False, debug=False, num_devices=N_CORES,
        num_swdge_queues=1,
    )
    emb = nc.dram_tensor("emb", [VOCAB, DIM], F32, kind="ExternalInput")
    idx_d = nc.dram_tensor("idx32", [P, COLS], I32, kind="ExternalInput")
    coef_d = nc.dram_tensor("coefs", [P, CW_TOT], F32, kind="ExternalInput")
    out_d = nc.dram_tensor("out", [P, OUTW], F32, kind="ExternalOutput")

    from contextlib import ExitStack

    with ExitStack() as ctx:
        sb = lambda name, shape, dt=F32: ctx.enter_context(
            nc.sbuf_tensor(name, shape, dt)
        )
        idx_sb = sb("idx_sb", [P, COLS], I32)
        coef_sb = sb("coef_sb", [P, CW_TOT])
        rows = sb("rows", [P, RCOLS, DIM])
        s_act = sb("s_act", [P, DIM])
        s_dve = sb("s_dve", [P, DIM])
        obuf = sb("obuf", [P, OUTW])
        sem = lambda name: ctx.enter_context(nc.semaphore(name))
        io_i, io_c = sem("io_i"), sem("io_c")
        gs = [sem(f"gs{j}") for j in range(COLS)]
        act_s, dve_s, io2 = sem("act_s"), sem("dve_s"), sem("io2")
        ramp = coef_sb[:, 0:CW_RAMP]

        # idx first (gates the gathers), coefs second; scalar engine HWDGE
        # dispatches earliest of the DMA-capable engines.
        nc.scalar.dma_start(idx_sb[:, :], idx_d[:, :]).then_inc(io_i, 16)
        nc.scalar.dma_start(coef_sb[:, :], coef_d[:, :]).then_inc(io_c, 16)
        # Pull the Abs table load into the idx-wait window (output junk).
        nc.scalar.activation(s_act[:, 0:1], s_dve[:, 0:1], AF.Abs)

        blk_ctx = nc.Block()
        block = blk_ctx.__enter__()

        @block.gpsimd
        def _(gpsimd: bass.BassGpSimd):
            gpsimd.wait_ge(io_i, 16)
            # 8 row gathers, then 2 neu element gathers (flat idx id*DIM+dim
            # lands t straight into the output buffer).
            for j in range(RCOLS):
                gpsimd.indirect_dma_start(
                    out=rows[:, j, :],
                    out_offset=None,
                    in_=emb[:, :],
                    in_offset=bass.IndirectOffsetOnAxis(
                        ap=idx_sb[:, j : j + 1], axis=0
                    ),
                ).then_inc(gs[j], 16)
            for j in range(RCOLS, COLS):
                gpsimd.indirect_dma_start(
                    out=obuf[:, j : j + 1],
                    out_offset=None,
                    in_=emb[:, :],
                    in_offset=bass.IndirectOffsetOnAxis(
                        ap=idx_sb[:, j : j + 1], axis=1
                    ),
                ).then_inc(gs[j], 16)

        @block.scalar
        def _(scalar: bass.BassEngine):
            for j in range(RCOLS):
                scalar.wait_ge(gs[j], 16)
                nc.scalar.activation(
                    s_act[:, :], rows[:, j, :], AF.Abs,
                    accum_out=obuf[:, COLS + j : COLS + j + 1],
                ).then_inc(act_s, 1)
            # Same-engine read of the accum columns guarantees the
            # READ_ACCUMULATOR writebacks are visible before the out-DMA
            # gate (act_s >= RCOLS+1).
            nc.scalar.activation(
                s_act[:, 0:RCOLS], obuf[:, COLS:OUTW], AF.Abs,
            ).then_inc(act_s, 1)

        @block.vector
        def _(vector: bass.BassEngine):
            vector.wait_ge(io_c, 16)
            for j in range(RCOLS):
                vector.wait_ge(gs[j], 16)
                nc.vector.scalar_tensor_tensor(
                    out=s_dve[:, :],
                    in0=ramp,
                    scalar=coef_sb[:, CW_RAMP + j : CW_RAMP + j + 1],
                    in1=rows[:, j, :],
                    op0=OP.is_equal,
                    op1=OP.mult,
                    accum_out=obuf[:, j : j + 1],
                ).then_inc(dve_s, 1)
            nc.vector.tensor_copy(
                s_dve[:, 0:RCOLS], obuf[:, 0:RCOLS]
            ).then_inc(dve_s, 1)

        @block.sync
        def _(sync: bass.BassEngine):
            sync.wait_ge(act_s, RCOLS + 1)
            sync.wait_ge(dve_s, RCOLS + 1)
            for j in range(RCOLS, COLS):
                sync.wait_ge(gs[j], 16)
            sync.dma_start(out_d[:, :], obuf[:, :]).then_inc(io2, 16)
            sync.wait_ge(io2, 16)

        blk_ctx.__exit__(None, None, None)
        ksr = nc._kernel_sem_range
        mono_start = ksr.start + 3 + (
            1 if nc._bir_kernel_barrier_sem is not None else 0
        )
        user_range = range(mono_start + len(nc._monotonic_sems), ksr.stop)
        nc.gpsimd.sem_clear(user_range)

    nc.compile()
    _nc_cache = nc
    return nc


def _deal(pos_ids, pos_dims, neg_ids, neg_dims, neu_ids, neu_dims):
    """Slot j of core c = constraint c + 8*j, so cols 0-3 pos, 4-7 neg,
    8-9 neu. Returns per-core (idx32 [128,10], coefs [128,520])."""
    ids = np.concatenate([pos_ids, neg_ids, neu_ids]).astype(np.int64)
    dims = np.concatenate([pos_dims, neg_dims, neu_dims]).astype(np.int64)

    idx32, coefs = [], []
    for c in range(N_CORES):
        g = np.arange(SLOTS) * N_CORES + c
        cid, cdim = ids[g].copy(), dims[g]
        # neu slots: flat element index
        cid[RCOLS * P :] = cid[RCOLS * P :] * DIM + cdim[RCOLS * P :]
        ix = np.ascontiguousarray(cid.astype(np.int32).reshape(COLS, P).T)
        cf = np.zeros((P, CW_TOT), np.float32)
        cf[:, 0:CW_RAMP] = np.arange(DIM, dtype=np.float32)[None, :]
        cf[:, CW_RAMP:] = cdim[: RCOLS * P].reshape(RCOLS, P).T
        idx32.append(ix)
        coefs.append(cf)
    return idx32, coefs


def _make_in_maps(emb, pos_ids, pos_dims, neg_ids, neg_dims, neu_ids, neu_dims):
    idx32, coefs = _deal(pos_ids, pos_dims, neg_ids, neg_dims, neu_ids, neu_dims)
    return [
        {"emb": emb, "idx32": idx32[c], "coefs": coefs[c]}
        for c in range(N_CORES)
    ]


def _finish_host(results):
    """results: per-core out [128, 18]. O(slots) exact finish."""
    total = 0.0
    for r in results:
        o = np.asarray(r["out"], dtype=np.float64)
        t = o[:, 0:COLS]
        rs = o[:, COLS:OUTW]
        tp, tn, tu = t[:, 0:4], t[:, 4:8], t[:, 8:10]
        at_p, at_n = np.abs(tp), np.abs(tn)
        pos_sign = np.where(tp <= 0, at_p + 0.1, -tp * 0.1).sum()
        neg_sign = np.where(tn >= 0, at_n + 0.1, -at_n * 0.1).sum()
        neu = 2.0 * np.abs(tu).sum()
        sparsity = C_SP * (rs.sum() - at_p.sum() - at_n.sum())
        total += pos_sign + neg_sign + neu + sparsity
    return np.asarray(total * CONSISTENCY_WEIGHT / N_ALL, dtype=np.float32)


def kernel(**inputs):
    emb = np.ascontiguousarray(np.asarray(inputs["embeddings"], dtype=np.float32))
    ids = {
        k: np.asarray(inputs[k]).astype(np.int64)
        for k in ("pos_ids", "pos_dims", "neg_ids", "neg_dims", "neu_ids", "neu_dims")
    }
    nc = _build_program()
    in_maps = _make_in_maps(
        emb, ids["pos_ids"], ids["pos_dims"], ids["neg_ids"], ids["neg_dims"],
        ids["neu_ids"], ids["neu_dims"],
    )
    res = run_bass_kernel_spmd(nc, in_maps, list(range(N_CORES)))
    return _finish_host(res.results)
